# revision 3
# baseline (speedup 1.0000x reference)
"""Trainium2 Bass kernel for the Koopman-operator rollout.

Reference computation: y0 = x[:, 0, :]  (shape [2048, 256]);
    y_t = y_{t-1} @ W.T  for t = 1..512, Y[:, t-1, :] = y_t.
Output: [2048, 512, 256] fp32 (1 GiB) -> memory-bound target.

Strategy (8 cores, data-parallel over batch, 256 rows/core):
  Let Wt = W.T.  Y[:, t] = y0 @ Wt^{t+1}.
  * Precompute P_j = Wt^j for j=1..16 via a log-depth product tree.
    Products use the duality Q_j = W^j = (P_j)^T so every product is
    expressible as matmul(out = lhsT.T @ rhs) with natural layouts.
  * Checkpoint states Z_i = y0 @ Wt^{16 i} (i=0..31), kept TRANSPOSED
    (k on partitions) so they can serve as matmul operands. Computed by
    prefix-doubling jumps A_m = Wt^{16 m} (m=1,2,4,8,16) -> rounding
    depth O(log T) instead of 512.
  * Per checkpoint i: Y[:, 16i+j-1] = Z_i @ P_j for j=1..16, as dense
    N=512 matmuls with Z_i^T stationary; PSUM -> SBUF copies on
    DVE/ACT; 2 MiB HWDGE DMAs to HBM.
  Matmul operands are bitcast to float32r (full PE rate at N>=256,
  ~tf32 multiply precision, fp32 PSUM accumulation).
"""

import os

import numpy as np

import concourse.bass as bass
import concourse.mybir as mybir
import concourse.tile as tile
from concourse import bacc
from concourse.bass import ds
from concourse.bass_utils import run_bass_kernel_spmd
from concourse.masks import make_identity

F32 = mybir.dt.float32
F32R = mybir.dt.float32r

N_CORES = 8
B_FULL = 2048
B_SH = B_FULL // N_CORES  # 256 batch rows per core
K = 256  # state dim
T = 512  # time steps
S = 16  # timesteps per checkpoint chunk
M = T // S  # 32 checkpoints

# engine choice for PSUM->SBUF output copies: every 3rd tile on ScalarE
ACT_COPY_EVERY = 3


def _mm(nc, out, lhsT, rhs, start, stop):
    # operands are float32r tiles already (producers round to f32r)
    nc.tensor.matmul(out, lhsT, rhs, start=start, stop=stop)


class _Mat:
    """A 256x256 matrix stored as an SBUF tile [128, 2, 256]:
    elem (p, h, c) = M[h*128 + p, c]."""

    def __init__(self, ap):
        self.ap = ap

    def half(self, hm):
        # [128, 256] slice: rows hm*128 .. hm*128+127 (partition = row)
        return self.ap[:, hm, :]

    def blk(self, hm, hc):
        # [128, 128] block: rows hm*128.., cols hc*128..
        return self.ap[:, hm, ds(128 * hc, 128)]


def _product(nc, psum_pool, dst, lhsT_mat, rhs_mat):
    """dst = lhsT_mat.T @ rhs_mat  (all 256x256 _Mats)."""
    for ha in range(2):
        ps = psum_pool.tile([128, 256], F32, tag="psz", name=f"psz_{ha}")
        for hm in range(2):
            _mm(nc, ps, lhsT_mat.blk(hm, ha), rhs_mat.half(hm), hm == 0, hm == 1)
        nc.vector.tensor_copy(dst.half(ha), ps)


def _build_program():
    nc = bacc.Bacc(
        "TRN2",
        target_bir_lowering=False,
        debug=False,
        enable_asserts=False,
        num_devices=N_CORES,
    )
    x_d = nc.dram_tensor("x", [B_SH, K], F32, kind="ExternalInput").ap()
    w_d = nc.dram_tensor("w", [K, K], F32, kind="ExternalInput").ap()
    y_d = nc.dram_tensor("y", [B_SH, T, K], F32, kind="ExternalOutput").ap()

    with tile.TileContext(nc) as tc:
        with (
            tc.tile_pool(name="consts", bufs=1) as consts,
            tc.tile_pool(name="mats", bufs=1) as mats,
            tc.tile_pool(name="zts", bufs=1) as zts,
            tc.tile_pool(name="ostage", bufs=3) as ostage,
            tc.tile_pool(name="pso", bufs=6, space="PSUM") as pso,
            tc.tile_pool(name="psz", bufs=2, space="PSUM") as psz,
        ):
            ident = consts.tile([128, 128], F32, tag="ident", name="ident")
            make_identity(nc, ident)

            w_nat = consts.tile([128, 2, K], F32, tag="w_nat", name="w_nat")
            x_nat = consts.tile([128, 2, K], F32, tag="x_nat", name="x_nat")
            for h in range(2):
                nc.sync.dma_start(out=w_nat[:, h, :], in_=w_d[ds(128 * h, 128), :])
                nc.sync.dma_start(out=x_nat[:, h, :], in_=x_d[ds(128 * h, 128), :])

            # Pcat holds P_1..P_16 row-half-major: [128, 2, 16*256]
            pcat = mats.tile([128, 2, S * K], F32R, tag="pcat", name="pcat")

            def P(j):  # 1-indexed power as a _Mat-like view
                class V:
                    def half(self, hm, _j=j):
                        return pcat[:, hm, ds(K * (_j - 1), K)]

                    def blk(self, hm, hc, _j=j):
                        return pcat[:, hm, ds(K * (_j - 1) + 128 * hc, 128)]

                return V()

            w_r = consts.tile([128, 2, K], F32R, tag="w_r", name="w_r")
            for h in range(2):
                nc.vector.tensor_copy(w_r[:, h, :], w_nat[:, h, :])
            q1 = _Mat(w_r)  # Q_1 = W (natural layout, rounded to f32r)

            # --- transposes: Z0^T = x^T, P_1 = W^T (PE transpose via identity)
            zt = [None] * M
            zt[0] = _Mat(zts.tile([128, 2, K], F32R, tag="zt0", name="zt0"))
            p1 = P(1)
            for g in range(2):
                for h in range(2):
                    pst = psz.tile([128, 128], F32, tag="psz", name=f"pst_{g}_{h}")
                    nc.tensor.transpose(pst, x_nat[:, g, ds(128 * h, 128)], ident)
                    nc.vector.tensor_copy(zt[0].ap[:, h, ds(128 * g, 128)], pst)
            for g in range(2):
                for h in range(2):
                    pst2 = psz.tile([128, 128], F32, tag="psz", name=f"pstw_{g}_{h}")
                    nc.tensor.transpose(pst2, w_nat[:, g, ds(128 * h, 128)], ident)
                    nc.vector.tensor_copy(pcat[:, h, ds(128 * g, 128)], pst2)

            # --- P-tree: P_1..P_16 (+ Q_2, Q_4, Q_8)
            def mk(tag):
                return _Mat(mats.tile([128, 2, K], F32R, tag=tag, name=tag))

            q2, q4, q8 = mk("q2"), mk("q4"), mk("q8")
            _product(nc, psz, P(2), q1, p1)  # P2 = Q1.T @ P1 = Wt^2
            _product(nc, psz, q2, p1, q1)  # Q2 = P1.T @ Q1 = W^2
            _product(nc, psz, P(3), q1, P(2))
            _product(nc, psz, P(4), q2, P(2))
            _product(nc, psz, q4, P(2), q2)
            for j in range(1, 5):
                _product(nc, psz, P(4 + j), q4, P(j))
            _product(nc, psz, q8, P(4), q4)
            for j in range(1, 9):
                _product(nc, psz, P(8 + j), q8, P(j))

            # --- A-ladder: A_m = Wt^{16 m} for m=1,2,4,8,16 (A_1 = P_16)
            q16 = mk("q16")
            _product(nc, psz, q16, P(8), q8)  # W^16
            a2, a4, a8, a16 = mk("a2"), mk("a4"), mk("a8"), mk("a16")
            qlad_a = mk("qlad_a")  # Q32, then Q128
            qlad_b = mk("qlad_b")  # Q64
            a1 = P(16)
            _product(nc, psz, a2, q16, a1)  # Wt^32
            _product(nc, psz, qlad_a, a1, q16)  # W^32
            _product(nc, psz, a4, qlad_a, a2)  # Wt^64
            _product(nc, psz, qlad_b, a2, qlad_a)  # W^64
            _product(nc, psz, a8, qlad_b, a4)  # Wt^128
            _product(nc, psz, qlad_a, a4, qlad_b)  # W^128 (reuse slot)
            _product(nc, psz, a16, qlad_a, a8)  # Wt^256
            amat = {1: a1, 2: a2, 4: a4, 8: a8, 16: a16}

            # --- checkpoint Z-tree (prefix doubling) interleaved with outputs
            copy_ctr = [0]

            def emit_outputs(i):
                """Y[:, 16i + j - 1, :] = Z_i @ P_j, j=1..16."""
                for m in range(2):  # batch half
                    ost = ostage.tile(
                        [128, S, K], F32, tag="ost", name=f"ost_{i}_{m}"
                    )
                    pos = []
                    for n in range(8):
                        po = pso.tile(
                            [128, 2, K], F32, tag="pso", name=f"pso_{i}_{m}_{n}"
                        )
                        pos.append(po)
                    for hm in range(2):
                        lhsT = zt[i].ap[:, hm, ds(128 * m, 128)]
                        for n in range(8):
                            # rhs: P_{2n+1}, P_{2n+2} concatenated = 512 cols
                            rhs = pcat[:, hm, ds(512 * n, 512)]
                            _mm(nc, pos[n], lhsT, rhs, hm == 0, hm == 1)
                    for n in range(8):
                        dst = ost[:, ds(2 * n, 2), :]
                        if copy_ctr[0] % ACT_COPY_EVERY == ACT_COPY_EVERY - 1:
                            nc.scalar.copy(dst, pos[n])
                        else:
                            nc.vector.tensor_copy(dst, pos[n])
                        copy_ctr[0] += 1
                    nc.sync.dma_start(
                        out=y_d[ds(128 * m, 128), ds(S * i, S), :], in_=ost
                    )

            def emit_zjump(dst_i, src_i, m):
                zt[dst_i] = _Mat(
                    zts.tile([128, 2, K], F32R, tag=f"zt{dst_i}", name=f"zt{dst_i}")
                )
                # Z_{dst}^T = A_m^T @ Z_{src}^T
                _product(nc, psz, zt[dst_i], amat[m], zt[src_i])

            emit_outputs(0)
            emit_zjump(16, 0, 16)
            emit_outputs(16)
            emit_zjump(8, 0, 8)
            emit_zjump(24, 16, 8)
            emit_outputs(8)
            emit_outputs(24)
            for src in (0, 8, 16, 24):
                emit_zjump(src + 4, src, 4)
            for src in (4, 12, 20, 28):
                emit_outputs(src)
            for src in (0, 4, 8, 12, 16, 20, 24, 28):
                emit_zjump(src + 2, src, 2)
            for src in (2, 6, 10, 14, 18, 22, 26, 30):
                emit_outputs(src)
            for src in range(0, 31, 2):
                emit_zjump(src + 1, src, 1)
            for src in range(1, 32, 2):
                emit_outputs(src)

    nc.compile()
    return nc


_cached_nc = None
_last_results = None


def kernel(x, W, T=None):
    global _cached_nc, _last_results
    if _cached_nc is None:
        _cached_nc = _build_program()
    nc = _cached_nc

    x2 = np.ascontiguousarray(np.asarray(x, dtype=np.float32).reshape(B_FULL, K))
    w2 = np.ascontiguousarray(np.asarray(W, dtype=np.float32))
    in_maps = [
        {"x": x2[i * B_SH : (i + 1) * B_SH], "w": w2} for i in range(N_CORES)
    ]
    res = run_bass_kernel_spmd(
        nc,
        in_maps,
        core_ids=list(range(N_CORES)),
        trace=bool(os.environ.get("BASS_TRACE")),
    )
    _last_results = res
    y = np.concatenate([res.results[i]["y"] for i in range(N_CORES)], axis=0)
    return y
